# revision 6
# baseline (speedup 1.0000x reference)
"""Trainium2 Bass kernel for nn_ApsMultiheadAttention (L=1024, N=8, E=1024, H=16).

Strategy: data-parallel over batch N=8 (one batch element per NeuronCore).
All heavy matmuls use float32r (full-rate) with host-pre-transposed weights so
every matmul operand has its contraction dim on partitions natively.

Per-core pipeline:
  phase 1: in_proj.
    Q/K rows computed transposed:  QKT[j, l] = sum_e WT[e,j] * xT[e,l]
    V rows computed natural:       V[s, jv]  = sum_e xT[e,s] * WTv[e,jv]
    V stored bf16 augmented with a ones column per head (for softmax denom).
  phase 2: attention per head h in "ST" layout:
    ST[s,l] = K_h Q_h^T   (lhsT = KT_h chunk, rhs = QT_h)   f32r
    expST = exp(ST/8)  (ScalarE, bf16 out)
    PV:  psum[0:64, l] = context_h^T,  psum[64, l] = den[l]  (ones column)
    context_h^T = psum * (1/den) broadcast; head-mean attn accumulated in bf16.
  phase 3: out_proj (contextT as lhsT, host-transposed out_w as rhs) + bias;
    attn accumulator (s-major) PE-transposed to [l, s] and DMA'd out.
"""

import os
import sys

import numpy as np

sys.path.insert(0, "/opt/trn_rl_repo")

import ml_dtypes  # noqa: E402

L, N, E, H = 1024, 8, 1024, 16
D = E // H  # 64
P = 128
EC = E // P  # 8 e-chunks
LC = L // P  # 8 l/s-chunks
JC_QK = 2 * E // P  # 16 chunks of Q,K rows

_CACHE = {}


def _build():
    import concourse.bass as bass
    import concourse.tile as tile
    from concourse import bacc, mybir
    from concourse.masks import make_identity
    from contextlib import ExitStack

    f32 = mybir.dt.float32
    f16 = mybir.dt.float16
    bf16 = mybir.dt.bfloat16
    EXP = mybir.ActivationFunctionType.Exp
    ADD = mybir.AluOpType.add
    MULT = mybir.AluOpType.mult

    nc = bacc.Bacc("TRN2", target_bir_lowering=False, debug=False, num_devices=8)

    xt_d = nc.dram_tensor("xt", [E, L], f16, kind="ExternalInput").ap()
    wt_d = nc.dram_tensor("wt", [E, 3 * E], f16, kind="ExternalInput").ap()
    bqk_d = nc.dram_tensor("bqk", [P, JC_QK], f32, kind="ExternalInput").ap()
    bv_d = nc.dram_tensor("bv", [1, E], f16, kind="ExternalInput").ap()
    owt_d = nc.dram_tensor("owt", [E, E], f16, kind="ExternalInput").ap()
    ob_d = nc.dram_tensor("ob", [1, E], f32, kind="ExternalInput").ap()
    ctx_d = nc.dram_tensor("ctx_out", [L, E], f32, kind="ExternalOutput").ap()
    attn_d = nc.dram_tensor("attn_out", [L, L], f32, kind="ExternalOutput").ap()
    invscr_d = nc.dram_tensor("inv_scratch", [H, L], f16).ap()

    with tile.TileContext(nc) as tc, ExitStack() as top, nc.allow_low_precision(
        reason="bf16 softmax-weight path is within the 2e-2 rel-err budget"
    ):
        # ---------- persistent pools ----------
        pers = top.enter_context(tc.tile_pool(name="pers", bufs=1))
        ctxT = pers.tile([P, EC, L], f16)  # context^T: [e_in, e_out, l]
        attn_acc = pers.tile([P, LC, L], f16)  # [s_in, s_out, l]
        ident = pers.tile([P, P], f16)
        outbc = pers.tile([P, E], f32)  # out bias broadcast over partitions
        bvbc = pers.tile([P, E], f16)  # v bias broadcast
        bqk_sb = pers.tile([P, JC_QK], f32)

        make_identity(nc, ident[:])
        nc.sync.dma_start(outbc[:], ob_d[0:1, :].to_broadcast((P, E)))
        nc.sync.dma_start(bvbc[:], bv_d[0:1, :].to_broadcast((P, E)))
        nc.sync.dma_start(bqk_sb[:], bqk_d[:, :])

        stage_a = top.enter_context(tc.tile_pool(name="stage_a", bufs=1))
        qkt = stage_a.tile([P, JC_QK, L], f16)  # [j_in, j_out, l]
        vaug = stage_a.tile([P, LC, H, D + 1], f16)  # [s_in, s_out, h, d|one]

        # ones column for the denominator trick
        nc.vector.memset(vaug[:, :, :, D : D + 1], 1.0)

        # ---------- phase 1: in_proj ----------
        with ExitStack() as ph1:
            xpool = ph1.enter_context(tc.tile_pool(name="xt", bufs=1))
            xt = xpool.tile([P, EC, L], f16)
            nc.sync.dma_start(xt[:], xt_d.rearrange("(eo p) l -> p eo l", p=P))

            with ExitStack() as ph1b:
                wpool = ph1b.enter_context(tc.tile_pool(name="wqk", bufs=2))
                pspool = ph1b.enter_context(
                    tc.tile_pool(name="ps1", bufs=2, space="PSUM")
                )
                for jc in range(JC_QK):
                    wt_sb = wpool.tile([P, EC, P], f16, tag="wqk")
                    nc.sync.dma_start(
                        wt_sb[:],
                        wt_d[:, jc * P : (jc + 1) * P].rearrange(
                            "(eo p) j -> p eo j", p=P
                        ),
                    )
                    for lh in range(2):
                        ps = pspool.tile([P, 512], f32, tag="ps1")
                        for ec in range(EC):
                            nc.tensor.matmul(
                                ps[:],
                                lhsT=wt_sb[:, ec, :],
                                rhs=xt[:, ec, lh * 512 : (lh + 1) * 512],
                                start=(ec == 0),
                                stop=(ec == EC - 1),
                            )
                        nc.vector.tensor_scalar_add(
                            qkt[:, jc, lh * 512 : (lh + 1) * 512],
                            ps[:],
                            bqk_sb[:, jc : jc + 1],
                        )

            with ExitStack() as ph1c:
                wvpool = ph1c.enter_context(tc.tile_pool(name="wv", bufs=1))
                pspool = ph1c.enter_context(
                    tc.tile_pool(name="ps1v", bufs=2, space="PSUM")
                )
                for vh in range(2):
                    wv_sb = wvpool.tile([P, EC, 512], f16, tag="wv")
                    nc.sync.dma_start(
                        wv_sb[:],
                        wt_d[:, 2 * E + vh * 512 : 2 * E + (vh + 1) * 512].rearrange(
                            "(eo p) j -> p eo j", p=P
                        ),
                    )
                    for sc in range(LC):
                        ps = pspool.tile([P, 512], f32, tag="ps1v")
                        for ec in range(EC):
                            nc.tensor.matmul(
                                ps[:],
                                lhsT=xt[:, ec, sc * P : (sc + 1) * P],
                                rhs=wv_sb[:, ec, :],
                                start=(ec == 0),
                                stop=(ec == EC - 1),
                            )
                        # scatter 8 head-blocks [P, 64] into vaug (bf16 cast)
                        nc.vector.tensor_copy(
                            vaug[:, sc, vh * 8 : (vh + 1) * 8, 0:D],
                            ps[:].rearrange("p (h d) -> p h d", d=D),
                        )
                # v bias (zero in this problem, applied for generality)
                for sc in range(LC):
                    nc.vector.tensor_tensor(
                        vaug[:, sc, :, 0:D],
                        vaug[:, sc, :, 0:D],
                        bvbc[:].rearrange("p (h d) -> p h d", d=D),
                        ADD,
                    )

        # ---------- phase 2: attention per head ----------
        with ExitStack() as ph2:
            expool = ph2.enter_context(tc.tile_pool(name="expst", bufs=2))
            stps = ph2.enter_context(tc.tile_pool(name="stps", bufs=2, space="PSUM"))
            pvps = ph2.enter_context(tc.tile_pool(name="pvps", bufs=4, space="PSUM"))
            invpool = ph2.enter_context(tc.tile_pool(name="inv", bufs=2))
            tmppool = ph2.enter_context(tc.tile_pool(name="tmp", bufs=3))

            for h in range(H):
                pq = 64 * (h % 2)
                jq = h // 2
                jk = 8 + h // 2
                expst = expool.tile([P, LC, L], f16, tag="expst")
                for sc in range(LC):
                    stp = stps.tile([P, L], f32, tag="stps")
                    for lh in range(2):
                        nc.tensor.matmul(
                            stp[:, lh * 512 : (lh + 1) * 512],
                            lhsT=qkt[pq : pq + 64, jk, sc * P : (sc + 1) * P],
                            rhs=qkt[pq : pq + 64, jq, lh * 512 : (lh + 1) * 512],
                            start=True,
                            stop=True,
                        )
                    nc.scalar.activation(expst[:, sc, :], stp[:], EXP, scale=0.125)

                pv0 = pvps.tile([D + 1, 512], f32, tag="pvps")
                pv1 = pvps.tile([D + 1, 512], f32, tag="pvps")
                for sc in range(LC):
                    for lh, pv in enumerate((pv0, pv1)):
                        nc.tensor.matmul(
                            pv[:],
                            lhsT=vaug[:, sc, h, :],
                            rhs=expst[:, sc, lh * 512 : (lh + 1) * 512],
                            start=(sc == 0),
                            stop=(sc == LC - 1),
                        )

                # 1/den (bf16), broadcast across partitions via DMA
                invrow = invpool.tile([D + 1, L], f16, tag="invrow")
                invbc = invpool.tile([P, L], f16, tag="invbc")
                for lh, pv in enumerate((pv0, pv1)):
                    nc.vector.reciprocal(
                        invrow[D : D + 1, lh * 512 : (lh + 1) * 512], pv[D : D + 1, :]
                    )
                # SBUF partition-broadcast needs a DRAM bounce (stride-0
                # partition APs are only legal on the DRAM side of a DMA)
                nc.sync.dma_start(invscr_d[h : h + 1, :], invrow[D : D + 1, :])
                nc.sync.dma_start(
                    invbc[:], invscr_d[h : h + 1, :].to_broadcast((P, L))
                )

                # normalize context^T rows for this head
                for lh, pv in enumerate((pv0, pv1)):
                    nc.vector.tensor_tensor(
                        ctxT[pq : pq + 64, h // 2, lh * 512 : (lh + 1) * 512],
                        pv[0:D, :],
                        invbc[0:D, lh * 512 : (lh + 1) * 512],
                        MULT,
                    )

                # attn mean accumulation (bf16):
                #   attn_acc[s, l] += expst[s, l] * invbc[s(bcast), l] / H
                for sc in range(LC):
                    if h == 0:
                        nc.vector.scalar_tensor_tensor(
                            out=attn_acc[:, sc, :],
                            in0=expst[:, sc, :],
                            scalar=1.0 / H,
                            in1=invbc[:],
                            op0=MULT,
                            op1=MULT,
                        )
                    else:
                        tmp = tmppool.tile([P, L], f16, tag="tmp")
                        nc.vector.scalar_tensor_tensor(
                            out=tmp[:],
                            in0=expst[:, sc, :],
                            scalar=1.0 / H,
                            in1=invbc[:],
                            op0=MULT,
                            op1=MULT,
                        )
                        nc.vector.tensor_tensor(
                            attn_acc[:, sc, :], tmp[:], attn_acc[:, sc, :], ADD
                        )

        # ---------- phase 3: out_proj + attn transpose ----------
        with ExitStack() as ph3:
            owpool = ph3.enter_context(tc.tile_pool(name="owt", bufs=1))
            outps = ph3.enter_context(tc.tile_pool(name="outps", bufs=2, space="PSUM"))
            outpool = ph3.enter_context(tc.tile_pool(name="outsb", bufs=3))
            trps = ph3.enter_context(tc.tile_pool(name="trps", bufs=2, space="PSUM"))
            stgpool = ph3.enter_context(tc.tile_pool(name="stg", bufs=3))

            owt = owpool.tile([P, EC, E], f16)
            nc.sync.dma_start(owt[:], owt_d.rearrange("(eo p) j -> p eo j", p=P))

            for lc in range(LC):
                for eh in range(2):
                    ps = outps.tile([P, 512], f32, tag="outps")
                    for ec in range(EC):
                        nc.tensor.matmul(
                            ps[:],
                            lhsT=ctxT[:, ec, lc * P : (lc + 1) * P],
                            rhs=owt[:, ec, eh * 512 : (eh + 1) * 512],
                            start=(ec == 0),
                            stop=(ec == EC - 1),
                        )
                    osb = outpool.tile([P, 512], f32, tag="outsb")
                    nc.vector.tensor_tensor(
                        osb[:], ps[:], outbc[:, eh * 512 : (eh + 1) * 512], ADD
                    )
                    nc.sync.dma_start(
                        ctx_d[lc * P : (lc + 1) * P, eh * 512 : (eh + 1) * 512], osb[:]
                    )

            for sc in range(LC):
                for lc in range(LC):
                    tp = trps.tile([P, P], f16, tag="trps")
                    nc.tensor.transpose(
                        tp[:], attn_acc[:, sc, lc * P : (lc + 1) * P], ident[:]
                    )
                    stg = stgpool.tile([P, P], f32, tag="stg")
                    nc.vector.tensor_copy(stg[:], tp[:])
                    nc.sync.dma_start(
                        attn_d[lc * P : (lc + 1) * P, sc * P : (sc + 1) * P], stg[:]
                    )

    nc.compile()
    return nc


def _prep_in_maps(x, in_proj_weight, in_proj_bias, out_w, out_b):
    wt = np.ascontiguousarray(in_proj_weight.T).astype(np.float16)  # [E, 3E]
    bqk = np.ascontiguousarray(
        in_proj_bias[: 2 * E].reshape(JC_QK, P).T
    ).astype(np.float32)  # [P, JC_QK]
    bv = in_proj_bias[2 * E :].reshape(1, E).astype(np.float16)
    owt = np.ascontiguousarray(out_w.T).astype(np.float16)  # [E, E]
    ob = out_b.reshape(1, E).astype(np.float32)
    in_maps = []
    for n in range(N):
        xt = np.ascontiguousarray(x[:, n, :].T).astype(np.float16)  # [E, L]
        in_maps.append(
            {"xt": xt, "wt": wt, "bqk": bqk, "bv": bv, "owt": owt, "ob": ob}
        )
    return in_maps


def _run(inputs, trace=False, tmpdir=None):
    from concourse.bass_utils import run_bass_kernel_spmd

    if "nc" not in _CACHE:
        _CACHE["nc"] = _build()
    nc = _CACHE["nc"]
    in_maps = _prep_in_maps(**inputs)
    res = run_bass_kernel_spmd(
        nc, in_maps, core_ids=list(range(N)), trace=trace, tmpdir=tmpdir
    )
    context = np.empty((L, N, E), np.float32)
    attn = np.empty((N, L, L), np.float32)
    for n in range(N):
        context[:, n, :] = res.results[n]["ctx_out"]
        attn[n] = res.results[n]["attn_out"]
    return (context, attn), res


def kernel(x, in_proj_weight, in_proj_bias, out_w, out_b):
    (context, attn), _ = _run(
        dict(
            x=x,
            in_proj_weight=in_proj_weight,
            in_proj_bias=in_proj_bias,
            out_w=out_w,
            out_b=out_b,
        )
    )
    return context, attn


# revision 9
# speedup vs baseline: 1.2674x; 1.2674x over previous
"""Trainium2 Bass kernel for nn_ApsMultiheadAttention (L=1024, N=8, E=1024, H=16).

Strategy: data-parallel over batch N=8 (one batch element per NeuronCore).
All heavy matmuls use float32r (full-rate) with host-pre-transposed weights so
every matmul operand has its contraction dim on partitions natively.

Per-core pipeline:
  phase 1: in_proj.
    Q/K rows computed transposed:  QKT[j, l] = sum_e WT[e,j] * xT[e,l]
    V rows computed natural:       V[s, jv]  = sum_e xT[e,s] * WTv[e,jv]
    V stored bf16 augmented with a ones column per head (for softmax denom).
  phase 2: attention per head h in "ST" layout:
    ST[s,l] = K_h Q_h^T   (lhsT = KT_h chunk, rhs = QT_h)   f32r
    expST = exp(ST/8)  (ScalarE, bf16 out)
    PV:  psum[0:64, l] = context_h^T,  psum[64, l] = den[l]  (ones column)
    context_h^T = psum * (1/den) broadcast; head-mean attn accumulated in bf16.
  phase 3: out_proj (contextT as lhsT, host-transposed out_w as rhs) + bias;
    attn accumulator (s-major) PE-transposed to [l, s] and DMA'd out.
"""

import math
import os
import sys

import numpy as np

sys.path.insert(0, "/opt/trn_rl_repo")

import ml_dtypes  # noqa: E402

L, N, E, H = 1024, 8, 1024, 16
D = E // H  # 64
P = 128
EC = E // P  # 8 e-chunks
LC = L // P  # 8 l/s-chunks
JC_QK = 2 * E // P  # 16 chunks of Q,K rows

_CACHE = {}


def _build():
    import concourse.bass as bass
    import concourse.tile as tile
    from concourse import bacc, mybir
    from concourse.masks import make_identity
    from contextlib import ExitStack

    f32 = mybir.dt.float32
    f16 = mybir.dt.float16
    bf16 = mybir.dt.bfloat16
    EXP = mybir.ActivationFunctionType.Exp
    LN = mybir.ActivationFunctionType.Ln
    ADD = mybir.AluOpType.add
    MULT = mybir.AluOpType.mult

    nc = bacc.Bacc("TRN2", target_bir_lowering=False, debug=False, num_devices=8)

    xt_d = nc.dram_tensor("xt", [E, L], f16, kind="ExternalInput").ap()
    wt_d = nc.dram_tensor("wt", [E, 3 * E], f16, kind="ExternalInput").ap()
    bqk_d = nc.dram_tensor("bqk", [P, JC_QK], f32, kind="ExternalInput").ap()
    bv_d = nc.dram_tensor("bv", [1, E], f16, kind="ExternalInput").ap()
    owt_d = nc.dram_tensor("owt", [E, E], f16, kind="ExternalInput").ap()
    ob_d = nc.dram_tensor("ob", [1, E], f32, kind="ExternalInput").ap()
    ctx_d = nc.dram_tensor("ctx_out", [L, E], f32, kind="ExternalOutput").ap()
    attn_d = nc.dram_tensor("attn_out", [L, L], f32, kind="ExternalOutput").ap()
    invscr_d = nc.dram_tensor("inv_scratch", [H, L], f16).ap()

    with tile.TileContext(nc) as tc, ExitStack() as top, nc.allow_low_precision(
        reason="bf16 softmax-weight path is within the 2e-2 rel-err budget"
    ):
        # ---------- persistent pools ----------
        pers = top.enter_context(tc.tile_pool(name="pers", bufs=1))
        ctxT = pers.tile([P, EC, L], f16)  # context^T: [e_in, e_out, l]
        attn_acc = pers.tile([P, LC, L], f16)  # [s_in, s_out, l]
        ident = pers.tile([P, P], f16)
        outbc = pers.tile([P, E], f32)  # out bias broadcast over partitions
        bvbc = pers.tile([P, E], f16)  # v bias broadcast
        bqk_sb = pers.tile([P, JC_QK], f32)

        make_identity(nc, ident[:])
        nc.sync.dma_start(outbc[:], ob_d[0:1, :].to_broadcast((P, E)))
        nc.sync.dma_start(bvbc[:], bv_d[0:1, :].to_broadcast((P, E)))
        nc.sync.dma_start(bqk_sb[:], bqk_d[:, :])

        stage_a = top.enter_context(tc.tile_pool(name="stage_a", bufs=1))
        qkt = stage_a.tile([P, JC_QK, L], f16)  # [j_in, j_out, l]
        vaug = stage_a.tile([P, LC, H, D + 1], f16)  # [s_in, s_out, h, d|one]

        # ones column for the denominator trick
        nc.vector.memset(vaug[:, :, :, D : D + 1], 1.0)

        # ---------- phase 1: in_proj ----------
        with ExitStack() as ph1:
            xpool = ph1.enter_context(tc.tile_pool(name="xt", bufs=1))
            xt = xpool.tile([P, EC, L], f16)
            nc.sync.dma_start(xt[:], xt_d.rearrange("(eo p) l -> p eo l", p=P))

            with ExitStack() as ph1c:
                wvpool = ph1c.enter_context(tc.tile_pool(name="wv", bufs=1))
                pspool = ph1c.enter_context(
                    tc.tile_pool(name="ps1v", bufs=2, space="PSUM")
                )
                for vh in range(2):
                    wv_sb = wvpool.tile([P, EC, 512], f16, tag="wv")
                    nc.sync.dma_start(
                        wv_sb[:],
                        wt_d[:, 2 * E + vh * 512 : 2 * E + (vh + 1) * 512].rearrange(
                            "(eo p) j -> p eo j", p=P
                        ),
                    )
                    for sc in range(LC):
                        ps = pspool.tile([P, 512], f32, tag="ps1v")
                        for ec in range(EC):
                            nc.tensor.matmul(
                                ps[:],
                                lhsT=xt[:, ec, sc * P : (sc + 1) * P],
                                rhs=wv_sb[:, ec, :],
                                start=(ec == 0),
                                stop=(ec == EC - 1),
                            )
                        # scatter 8 head-blocks [P, 64] into vaug (bf16 cast)
                        nc.vector.tensor_copy(
                            vaug[:, sc, vh * 8 : (vh + 1) * 8, 0:D],
                            ps[:].rearrange("p (h d) -> p h d", d=D),
                        )
                # v bias (zero in this problem, applied for generality)
                for sc in range(LC):
                    nc.vector.tensor_tensor(
                        vaug[:, sc, :, 0:D],
                        vaug[:, sc, :, 0:D],
                        bvbc[:].rearrange("p (h d) -> p h d", d=D),
                        ADD,
                    )

            with ExitStack() as ph1b:
                wpool = ph1b.enter_context(tc.tile_pool(name="wqk", bufs=2))
                pspool = ph1b.enter_context(
                    tc.tile_pool(name="ps1", bufs=2, space="PSUM")
                )
                for jc in [j for pair in zip(range(8), range(8, 16)) for j in pair]:
                    wt_sb = wpool.tile([P, EC, P], f16, tag="wqk")
                    nc.sync.dma_start(
                        wt_sb[:],
                        wt_d[:, jc * P : (jc + 1) * P].rearrange(
                            "(eo p) j -> p eo j", p=P
                        ),
                    )
                    for lh in range(2):
                        ps = pspool.tile([P, 512], f32, tag="ps1")
                        for ec in range(EC):
                            nc.tensor.matmul(
                                ps[:],
                                lhsT=wt_sb[:, ec, :],
                                rhs=xt[:, ec, lh * 512 : (lh + 1) * 512],
                                start=(ec == 0),
                                stop=(ec == EC - 1),
                            )
                        nc.vector.tensor_scalar_add(
                            qkt[:, jc, lh * 512 : (lh + 1) * 512],
                            ps[:],
                            bqk_sb[:, jc : jc + 1],
                        )

        # ---------- phase 2: attention per head ----------
        with ExitStack() as ph2:
            expool = ph2.enter_context(tc.tile_pool(name="expst", bufs=2))
            stps = ph2.enter_context(tc.tile_pool(name="stps", bufs=2, space="PSUM"))
            pvps = ph2.enter_context(tc.tile_pool(name="pvps", bufs=4, space="PSUM"))
            invpool = ph2.enter_context(tc.tile_pool(name="inv", bufs=2))
            tmppool = ph2.enter_context(tc.tile_pool(name="tmp", bufs=3))

            for h in range(H):
                pq = 64 * (h % 2)
                jq = h // 2
                jk = 8 + h // 2
                expst = expool.tile([P, LC, L], f16, tag="expst")
                for sc in range(LC):
                    stp = stps.tile([P, L], f32, tag="stps")
                    for lh in range(2):
                        nc.tensor.matmul(
                            stp[:, lh * 512 : (lh + 1) * 512],
                            lhsT=qkt[pq : pq + 64, jk, sc * P : (sc + 1) * P],
                            rhs=qkt[pq : pq + 64, jq, lh * 512 : (lh + 1) * 512],
                            start=True,
                            stop=True,
                        )
                    nc.scalar.activation(expst[:, sc, :], stp[:], EXP, scale=0.125)

                pv0 = pvps.tile([D + 1, 512], f32, tag="pvps")
                pv1 = pvps.tile([D + 1, 512], f32, tag="pvps")
                for sc in range(LC):
                    for lh, pv in enumerate((pv0, pv1)):
                        nc.tensor.matmul(
                            pv[:],
                            lhsT=vaug[:, sc, h, :],
                            rhs=expst[:, sc, lh * 512 : (lh + 1) * 512],
                            start=(sc == 0),
                            stop=(sc == LC - 1),
                        )

                # inv16 = 1/(16*den) = exp(-ln(den) - ln 16)  (ScalarE; the
                # natural_log_exp_and_others table set covers both funcs)
                lnrow = invpool.tile([D + 1, L], f32, tag="lnrow")
                invrow = invpool.tile([D + 1, L], f16, tag="invrow")
                invbc = invpool.tile([P, L], f16, tag="invbc")
                for lh, pv in enumerate((pv0, pv1)):
                    nc.scalar.activation(
                        lnrow[D : D + 1, lh * 512 : (lh + 1) * 512],
                        pv[D : D + 1, :],
                        LN,
                        scale=16.0,
                    )
                nc.scalar.activation(
                    invrow[D : D + 1, :],
                    lnrow[D : D + 1, :],
                    EXP,
                    scale=-1.0,
                )
                # SBUF partition-broadcast needs a DRAM bounce (stride-0
                # partition APs are only legal on the DRAM side of a DMA)
                nc.sync.dma_start(invscr_d[h : h + 1, :], invrow[D : D + 1, :])
                nc.sync.dma_start(
                    invbc[:], invscr_d[h : h + 1, :].to_broadcast((P, L))
                )

                # normalize context^T rows for this head
                for lh, pv in enumerate((pv0, pv1)):
                    nc.vector.tensor_tensor(
                        ctxT[pq : pq + 64, h // 2, lh * 512 : (lh + 1) * 512],
                        pv[0:D, :],
                        invbc[0:D, lh * 512 : (lh + 1) * 512],
                        MULT,
                    )

                # attn mean accumulation (f16, invbc already holds the /16):
                #   attn_acc[s, l] += expst[s, l] * invbc[bcast, l]
                # adds split across DVE (sc 0-3) and GpSimd (sc 4-7)
                for sc in range(LC):
                    if h == 0:
                        nc.vector.tensor_tensor(
                            attn_acc[:, sc, :], expst[:, sc, :], invbc[:], MULT
                        )
                    else:
                        tmp = tmppool.tile([P, L], f16, tag="tmp")
                        nc.vector.tensor_tensor(
                            tmp[:], expst[:, sc, :], invbc[:], MULT
                        )
                        eng = nc.vector if sc < 4 else nc.gpsimd
                        eng.tensor_tensor(
                            attn_acc[:, sc, :], tmp[:], attn_acc[:, sc, :], ADD
                        )

        # ---------- phase 3: out_proj + attn transpose ----------
        with ExitStack() as ph3:
            owpool = ph3.enter_context(tc.tile_pool(name="owt", bufs=1))
            outps = ph3.enter_context(tc.tile_pool(name="outps", bufs=2, space="PSUM"))
            outpool = ph3.enter_context(tc.tile_pool(name="outsb", bufs=3))
            trps = ph3.enter_context(tc.tile_pool(name="trps", bufs=2, space="PSUM"))
            stgpool = ph3.enter_context(tc.tile_pool(name="stg", bufs=3))

            owt = owpool.tile([P, EC, E], f16)
            nc.sync.dma_start(owt[:], owt_d.rearrange("(eo p) j -> p eo j", p=P))

            for lc in range(LC):
                for eh in range(2):
                    ps = outps.tile([P, 512], f32, tag="outps")
                    for ec in range(EC):
                        nc.tensor.matmul(
                            ps[:],
                            lhsT=ctxT[:, ec, lc * P : (lc + 1) * P],
                            rhs=owt[:, ec, eh * 512 : (eh + 1) * 512],
                            start=(ec == 0),
                            stop=(ec == EC - 1),
                        )
                    osb = outpool.tile([P, 512], f32, tag="outsb")
                    nc.vector.tensor_tensor(
                        osb[:], ps[:], outbc[:, eh * 512 : (eh + 1) * 512], ADD
                    )
                    nc.sync.dma_start(
                        ctx_d[lc * P : (lc + 1) * P, eh * 512 : (eh + 1) * 512], osb[:]
                    )

            for sc in range(LC):
                for lc in range(LC):
                    tp = trps.tile([P, P], f16, tag="trps")
                    nc.tensor.transpose(
                        tp[:], attn_acc[:, sc, lc * P : (lc + 1) * P], ident[:]
                    )
                    stg = stgpool.tile([P, P], f32, tag="stg")
                    nc.scalar.copy(stg[:], tp[:])
                    nc.sync.dma_start(
                        attn_d[lc * P : (lc + 1) * P, sc * P : (sc + 1) * P], stg[:]
                    )

    nc.compile()
    return nc


def _prep_in_maps(x, in_proj_weight, in_proj_bias, out_w, out_b):
    wt = np.ascontiguousarray(in_proj_weight.T).astype(np.float16)  # [E, 3E]
    bqk = np.ascontiguousarray(
        in_proj_bias[: 2 * E].reshape(JC_QK, P).T
    ).astype(np.float32)  # [P, JC_QK]
    bv = in_proj_bias[2 * E :].reshape(1, E).astype(np.float16)
    owt = np.ascontiguousarray(out_w.T * 16.0).astype(np.float16)  # [E, E] x16 compensates 1/16 in inv_den
    ob = out_b.reshape(1, E).astype(np.float32)
    in_maps = []
    for n in range(N):
        xt = np.ascontiguousarray(x[:, n, :].T).astype(np.float16)  # [E, L]
        in_maps.append(
            {"xt": xt, "wt": wt, "bqk": bqk, "bv": bv, "owt": owt, "ob": ob}
        )
    return in_maps


def _run(inputs, trace=False, tmpdir=None):
    from concourse.bass_utils import run_bass_kernel_spmd

    if "nc" not in _CACHE:
        _CACHE["nc"] = _build()
    nc = _CACHE["nc"]
    in_maps = _prep_in_maps(**inputs)
    res = run_bass_kernel_spmd(
        nc, in_maps, core_ids=list(range(N)), trace=trace, tmpdir=tmpdir
    )
    context = np.empty((L, N, E), np.float32)
    attn = np.empty((N, L, L), np.float32)
    for n in range(N):
        context[:, n, :] = res.results[n]["ctx_out"]
        attn[n] = res.results[n]["attn_out"]
    return (context, attn), res


def kernel(x, in_proj_weight, in_proj_bias, out_w, out_b):
    (context, attn), _ = _run(
        dict(
            x=x,
            in_proj_weight=in_proj_weight,
            in_proj_bias=in_proj_bias,
            out_w=out_w,
            out_b=out_b,
        )
    )
    return context, attn


# revision 11
# speedup vs baseline: 1.3132x; 1.0361x over previous
"""Trainium2 Bass kernel for nn_ApsMultiheadAttention (L=1024, N=8, E=1024, H=16).

Strategy: data-parallel over batch N=8 (one batch element per NeuronCore).
All heavy matmuls use float32r (full-rate) with host-pre-transposed weights so
every matmul operand has its contraction dim on partitions natively.

Per-core pipeline:
  phase 1: in_proj.
    Q/K rows computed transposed:  QKT[j, l] = sum_e WT[e,j] * xT[e,l]
    V rows computed natural:       V[s, jv]  = sum_e xT[e,s] * WTv[e,jv]
    V stored bf16 augmented with a ones column per head (for softmax denom).
  phase 2: attention per head h in "ST" layout:
    ST[s,l] = K_h Q_h^T   (lhsT = KT_h chunk, rhs = QT_h)   f32r
    expST = exp(ST/8)  (ScalarE, bf16 out)
    PV:  psum[0:64, l] = context_h^T,  psum[64, l] = den[l]  (ones column)
    context_h^T = psum * (1/den) broadcast; head-mean attn accumulated in bf16.
  phase 3: out_proj (contextT as lhsT, host-transposed out_w as rhs) + bias;
    attn accumulator (s-major) PE-transposed to [l, s] and DMA'd out.
"""

import math
import os
import sys

import numpy as np

sys.path.insert(0, "/opt/trn_rl_repo")

import ml_dtypes  # noqa: E402

L, N, E, H = 1024, 8, 1024, 16
D = E // H  # 64
P = 128
EC = E // P  # 8 e-chunks
LC = L // P  # 8 l/s-chunks
JC_QK = 2 * E // P  # 16 chunks of Q,K rows

_CACHE = {}


def _build():
    import concourse.bass as bass
    import concourse.tile as tile
    from concourse import bacc, mybir
    from concourse.masks import make_identity
    from contextlib import ExitStack

    f32 = mybir.dt.float32
    f16 = mybir.dt.float16
    bf16 = mybir.dt.bfloat16
    EXP = mybir.ActivationFunctionType.Exp
    LN = mybir.ActivationFunctionType.Ln
    IDENT = mybir.ActivationFunctionType.Identity
    ADD = mybir.AluOpType.add
    MULT = mybir.AluOpType.mult

    # Make Exp and Ln resolve to the single set that contains both, so the
    # table-load pass emits one load instead of thrashing between sets.
    import concourse.hw_specs as hw_specs_mod

    if not getattr(bacc, "_act_tables_patched", False):
        _orig_get_tables = bacc.get_activation_tables

        def _patched_get_tables(arch):
            tables = _orig_get_tables(arch)
            for name, funcs in tables.items():
                if name != "natural_log_exp_and_others":
                    funcs.discard(mybir.ActivationFunctionType.Exp)
                    funcs.discard(mybir.ActivationFunctionType.Ln)
            return tables

        bacc.get_activation_tables = _patched_get_tables
        bacc._act_tables_patched = True

    nc = bacc.Bacc("TRN2", target_bir_lowering=False, debug=False, num_devices=8)

    xt_d = nc.dram_tensor("xt", [E, L], f16, kind="ExternalInput").ap()
    wt_d = nc.dram_tensor("wt", [E, 3 * E], f16, kind="ExternalInput").ap()
    bqk_d = nc.dram_tensor("bqk", [P, JC_QK], f32, kind="ExternalInput").ap()
    bv_d = nc.dram_tensor("bv", [1, E], bf16, kind="ExternalInput").ap()
    owt_d = nc.dram_tensor("owt", [E, E], f16, kind="ExternalInput").ap()
    ob_d = nc.dram_tensor("ob", [1, E], f32, kind="ExternalInput").ap()
    ctx_d = nc.dram_tensor("ctx_out", [L, E], f32, kind="ExternalOutput").ap()
    attn_d = nc.dram_tensor("attn_out", [L, L], f32, kind="ExternalOutput").ap()
    invscr_d = nc.dram_tensor("inv_scratch", [H, L], bf16).ap()

    with tile.TileContext(nc) as tc, ExitStack() as top, nc.allow_low_precision(
        reason="bf16 softmax-weight path is within the 2e-2 rel-err budget"
    ):
        # ---------- persistent pools ----------
        pers = top.enter_context(tc.tile_pool(name="pers", bufs=1))
        ctxT = pers.tile([P, EC, L], f16)  # context^T: [e_in, e_out, l]
        attn_acc = pers.tile([P, LC, L], bf16)  # [s_in, s_out, l]
        ident = pers.tile([P, P], bf16)
        outbc = pers.tile([P, E], f32)  # out bias broadcast over partitions
        bvbc = pers.tile([P, E], bf16)  # v bias broadcast
        bqk_sb = pers.tile([P, JC_QK], f32)

        make_identity(nc, ident[:])
        nc.sync.dma_start(outbc[:], ob_d[0:1, :].to_broadcast((P, E)))
        nc.sync.dma_start(bvbc[:], bv_d[0:1, :].to_broadcast((P, E)))
        nc.sync.dma_start(bqk_sb[:], bqk_d[:, :])

        stage_a = top.enter_context(tc.tile_pool(name="stage_a", bufs=1))
        qkt = stage_a.tile([P, JC_QK, L], f16)  # [j_in, j_out, l]
        vaug = stage_a.tile([P, LC, H, D + 1], bf16)  # [s_in, s_out, h, d|one]

        # ones column for the denominator trick
        nc.vector.memset(vaug[:, :, :, D : D + 1], 1.0)

        # ---------- phase 1: in_proj ----------
        with ExitStack() as ph1:
            xpool = ph1.enter_context(tc.tile_pool(name="xt", bufs=1))
            xt = xpool.tile([P, EC, L], f16)
            nc.sync.dma_start(xt[:], xt_d.rearrange("(eo p) l -> p eo l", p=P))

            with ExitStack() as ph1c:
                wvpool = ph1c.enter_context(tc.tile_pool(name="wv", bufs=1))
                pspool = ph1c.enter_context(
                    tc.tile_pool(name="ps1v", bufs=2, space="PSUM")
                )
                for vh in range(2):
                    wv_sb = wvpool.tile([P, EC, 512], f16, tag="wv")
                    nc.sync.dma_start(
                        wv_sb[:],
                        wt_d[:, 2 * E + vh * 512 : 2 * E + (vh + 1) * 512].rearrange(
                            "(eo p) j -> p eo j", p=P
                        ),
                    )
                    for sc in range(LC):
                        ps = pspool.tile([P, 512], f32, tag="ps1v")
                        for ec in range(EC):
                            nc.tensor.matmul(
                                ps[:],
                                lhsT=xt[:, ec, sc * P : (sc + 1) * P],
                                rhs=wv_sb[:, ec, :],
                                start=(ec == 0),
                                stop=(ec == EC - 1),
                            )
                        # scatter 8 head-blocks [P, 64] into vaug (bf16 cast)
                        nc.vector.tensor_copy(
                            vaug[:, sc, vh * 8 : (vh + 1) * 8, 0:D],
                            ps[:].rearrange("p (h d) -> p h d", d=D),
                        )
                # v bias (zero in this problem, applied for generality)
                for sc in range(LC):
                    nc.vector.tensor_tensor(
                        vaug[:, sc, :, 0:D],
                        vaug[:, sc, :, 0:D],
                        bvbc[:].rearrange("p (h d) -> p h d", d=D),
                        ADD,
                    )

            with ExitStack() as ph1b:
                wpool = ph1b.enter_context(tc.tile_pool(name="wqk", bufs=2))
                pspool = ph1b.enter_context(
                    tc.tile_pool(name="ps1", bufs=2, space="PSUM")
                )
                for jc in [j for pair in zip(range(8), range(8, 16)) for j in pair]:
                    wt_sb = wpool.tile([P, EC, P], f16, tag="wqk")
                    nc.sync.dma_start(
                        wt_sb[:],
                        wt_d[:, jc * P : (jc + 1) * P].rearrange(
                            "(eo p) j -> p eo j", p=P
                        ),
                    )
                    for lh in range(2):
                        ps = pspool.tile([P, 512], f32, tag="ps1")
                        for ec in range(EC):
                            nc.tensor.matmul(
                                ps[:],
                                lhsT=wt_sb[:, ec, :],
                                rhs=xt[:, ec, lh * 512 : (lh + 1) * 512],
                                start=(ec == 0),
                                stop=(ec == EC - 1),
                            )
                        nc.scalar.activation(
                            qkt[:, jc, lh * 512 : (lh + 1) * 512],
                            ps[:],
                            IDENT,
                            bias=bqk_sb[:, jc : jc + 1],
                        )

        # ---------- phase 2: attention per head ----------
        with ExitStack() as ph2:
            expool = ph2.enter_context(tc.tile_pool(name="expst", bufs=2))
            stps = ph2.enter_context(tc.tile_pool(name="stps", bufs=2, space="PSUM"))
            pvps = ph2.enter_context(tc.tile_pool(name="pvps", bufs=4, space="PSUM"))
            invpool = ph2.enter_context(tc.tile_pool(name="inv", bufs=2))
            tmppool = ph2.enter_context(tc.tile_pool(name="tmp", bufs=3))

            for h in range(H):
                pq = 64 * (h % 2)
                jq = h // 2
                jk = 8 + h // 2
                expst = expool.tile([P, LC, L], bf16, tag="expst")
                for sc in range(LC):
                    stp = stps.tile([P, L], f32, tag="stps")
                    for lh in range(2):
                        nc.tensor.matmul(
                            stp[:, lh * 512 : (lh + 1) * 512],
                            lhsT=qkt[pq : pq + 64, jk, sc * P : (sc + 1) * P],
                            rhs=qkt[pq : pq + 64, jq, lh * 512 : (lh + 1) * 512],
                            start=True,
                            stop=True,
                        )
                    nc.scalar.activation(expst[:, sc, :], stp[:], EXP, scale=0.125)

                pv0 = pvps.tile([D + 1, 512], f32, tag="pvps")
                pv1 = pvps.tile([D + 1, 512], f32, tag="pvps")
                for sc in range(LC):
                    for lh, pv in enumerate((pv0, pv1)):
                        nc.tensor.matmul(
                            pv[:],
                            lhsT=vaug[:, sc, h, :],
                            rhs=expst[:, sc, lh * 512 : (lh + 1) * 512],
                            start=(sc == 0),
                            stop=(sc == LC - 1),
                        )

                # inv16 = 1/(16*den) = exp(-ln(den) - ln 16)  (ScalarE; the
                # natural_log_exp_and_others table set covers both funcs)
                lnrow = invpool.tile([D + 1, L], f32, tag="lnrow")
                invrow = invpool.tile([D + 1, L], bf16, tag="invrow")
                invbc = invpool.tile([P, L], bf16, tag="invbc")
                for lh, pv in enumerate((pv0, pv1)):
                    nc.scalar.activation(
                        lnrow[D : D + 1, lh * 512 : (lh + 1) * 512],
                        pv[D : D + 1, :],
                        LN,
                        scale=16.0,
                    )
                nc.scalar.activation(
                    invrow[D : D + 1, :],
                    lnrow[D : D + 1, :],
                    EXP,
                    scale=-1.0,
                )
                # SBUF partition-broadcast needs a DRAM bounce (stride-0
                # partition APs are only legal on the DRAM side of a DMA)
                nc.sync.dma_start(invscr_d[h : h + 1, :], invrow[D : D + 1, :])
                nc.sync.dma_start(
                    invbc[:], invscr_d[h : h + 1, :].to_broadcast((P, L))
                )

                # normalize context^T rows for this head
                for lh, pv in enumerate((pv0, pv1)):
                    nc.vector.tensor_tensor(
                        ctxT[pq : pq + 64, h // 2, lh * 512 : (lh + 1) * 512],
                        pv[0:D, :],
                        invbc[0:D, lh * 512 : (lh + 1) * 512],
                        MULT,
                    )

                # attn mean accumulation (f16, invbc already holds the /16):
                #   attn_acc[s, l] += expst[s, l] * invbc[bcast, l]
                # adds split across DVE (sc 0-3) and GpSimd (sc 4-7)
                for sc in range(LC):
                    if h == 0:
                        nc.vector.tensor_tensor(
                            attn_acc[:, sc, :], expst[:, sc, :], invbc[:], MULT
                        )
                    else:
                        tmp = tmppool.tile([P, L], bf16, tag="tmp")
                        nc.vector.tensor_tensor(
                            tmp[:], expst[:, sc, :], invbc[:], MULT
                        )
                        eng = nc.vector if sc < 4 else nc.gpsimd
                        eng.tensor_tensor(
                            attn_acc[:, sc, :], tmp[:], attn_acc[:, sc, :], ADD
                        )

        # ---------- phase 3: out_proj + attn transpose ----------
        with ExitStack() as ph3:
            owpool = ph3.enter_context(tc.tile_pool(name="owt", bufs=1))
            outps = ph3.enter_context(tc.tile_pool(name="outps", bufs=2, space="PSUM"))
            outpool = ph3.enter_context(tc.tile_pool(name="outsb", bufs=3))
            trps = ph3.enter_context(tc.tile_pool(name="trps", bufs=2, space="PSUM"))
            stgpool = ph3.enter_context(tc.tile_pool(name="stg", bufs=3))

            owt = owpool.tile([P, EC, E], f16)
            nc.sync.dma_start(owt[:], owt_d.rearrange("(eo p) j -> p eo j", p=P))

            for lc in range(LC):
                for eh in range(2):
                    ps = outps.tile([P, 512], f32, tag="outps")
                    for ec in range(EC):
                        nc.tensor.matmul(
                            ps[:],
                            lhsT=ctxT[:, ec, lc * P : (lc + 1) * P],
                            rhs=owt[:, ec, eh * 512 : (eh + 1) * 512],
                            start=(ec == 0),
                            stop=(ec == EC - 1),
                        )
                    osb = outpool.tile([P, 512], f32, tag="outsb")
                    nc.vector.tensor_tensor(
                        osb[:], ps[:], outbc[:, eh * 512 : (eh + 1) * 512], ADD
                    )
                    nc.sync.dma_start(
                        ctx_d[lc * P : (lc + 1) * P, eh * 512 : (eh + 1) * 512], osb[:]
                    )

            for sc in range(LC):
                for lc in range(LC):
                    tp = trps.tile([P, P], bf16, tag="trps")
                    nc.tensor.transpose(
                        tp[:], attn_acc[:, sc, lc * P : (lc + 1) * P], ident[:]
                    )
                    stg = stgpool.tile([P, P], f32, tag="stg")
                    nc.scalar.copy(stg[:], tp[:])
                    nc.sync.dma_start(
                        attn_d[lc * P : (lc + 1) * P, sc * P : (sc + 1) * P], stg[:]
                    )

    nc.compile()
    return nc


def _prep_in_maps(x, in_proj_weight, in_proj_bias, out_w, out_b):
    wt = np.ascontiguousarray(in_proj_weight.T).astype(np.float16)  # [E, 3E]
    bqk = np.ascontiguousarray(
        in_proj_bias[: 2 * E].reshape(JC_QK, P).T
    ).astype(np.float32)  # [P, JC_QK]
    bv = in_proj_bias[2 * E :].reshape(1, E).astype(ml_dtypes.bfloat16)
    owt = np.ascontiguousarray(out_w.T * 16.0).astype(np.float16)  # [E, E] x16 compensates 1/16 in inv_den
    ob = out_b.reshape(1, E).astype(np.float32)
    in_maps = []
    for n in range(N):
        xt = np.ascontiguousarray(x[:, n, :].T).astype(np.float16)  # [E, L]
        in_maps.append(
            {"xt": xt, "wt": wt, "bqk": bqk, "bv": bv, "owt": owt, "ob": ob}
        )
    return in_maps


def _run(inputs, trace=False, tmpdir=None):
    from concourse.bass_utils import run_bass_kernel_spmd

    if "nc" not in _CACHE:
        _CACHE["nc"] = _build()
    nc = _CACHE["nc"]
    in_maps = _prep_in_maps(**inputs)
    res = run_bass_kernel_spmd(
        nc, in_maps, core_ids=list(range(N)), trace=trace, tmpdir=tmpdir
    )
    context = np.empty((L, N, E), np.float32)
    attn = np.empty((N, L, L), np.float32)
    for n in range(N):
        context[:, n, :] = res.results[n]["ctx_out"]
        attn[n] = res.results[n]["attn_out"]
    return (context, attn), res


def kernel(x, in_proj_weight, in_proj_bias, out_w, out_b):
    (context, attn), _ = _run(
        dict(
            x=x,
            in_proj_weight=in_proj_weight,
            in_proj_bias=in_proj_bias,
            out_w=out_w,
            out_b=out_b,
        )
    )
    return context, attn
